# revision 9
# baseline (speedup 1.0000x reference)
"""nn_Attention_7765300871328 — Bass/Tile kernel for TRN2, 8 NeuronCores.

Sharding: tensor-parallel over heads. Core c handles group g=c//4 and 4 of the
16 heads of that group (its Wq slice is reordered so its own heads are slots
0-3; slots 4-15 are the rest of the group, used only for the top-k pooling
which needs softmax probabilities of all 16 heads of the group). Each core
computes a partial output projection over its 4 heads; the host sums the 8
partials. The program is identical on every core — only input data differs.
"""

import sys, types
import numpy as np
import ml_dtypes

T = 2048; HID = 2048; HQ = 32; G = 2; D = 64
KERNEL = 32; STRIDE = 16; BLOCK = 64; TOPK = 16
WINDOW = 512
C = (T - KERNEL) // STRIDE + 1           # 127
NB = T // BLOCK                          # 32
QT = T // 128                            # 16 query tiles
NEG = -1e30
SCALE = 0.125
BF16 = ml_dtypes.bfloat16

_CACHE = {}


def _install_ntff_hook():
    try:
        import antenv
        if "antenv.axon_hooks" in sys.modules:
            return
        mod = types.ModuleType("antenv.axon_hooks")
        mod._hook = None
        def set_hook(h): mod._hook = h
        def get_hook(): return mod._hook
        mod.set_axon_ntff_profile_hook = set_hook
        mod.get_axon_ntff_profile_hook = get_hook
        sys.modules["antenv.axon_hooks"] = mod
        antenv.axon_hooks = mod
        from trn_agent_boot.trn_boot import _ntff_profile_via_ctypes
        hook = _ntff_profile_via_ctypes('/opt/axon/libaxon_pjrt.so')
        if hook is not None:
            set_hook(hook)
    except Exception:
        pass


def _host_tables():
    if "tables" in _CACHE:
        return _CACHE["tables"]
    t = {}
    half = D // 2
    inv = 1.0 / (10000.0 ** (np.arange(half, dtype=np.float64) / half))
    freqs = np.arange(T, dtype=np.float64)[None, :] * inv[:, None]   # [32, T]
    cos = np.cos(freqs); sin = np.sin(freqs)
    T1q = np.empty((128, T), np.float32); T2q = np.empty((128, T), np.float32)
    T1q[0:32] = cos * SCALE; T1q[32:64] = cos * SCALE
    T2q[0:32] = -sin * SCALE; T2q[32:64] = sin * SCALE
    T1q[64:128] = T1q[0:64]; T2q[64:128] = T2q[0:64]
    T1k = np.zeros((128, T), np.float32); T2k = np.zeros((128, T), np.float32)
    T1k[64:96] = cos; T1k[96:128] = cos
    T2k[64:96] = -sin; T2k[96:128] = sin
    t["rT1q"] = T1q.astype(BF16); t["rT2q"] = T2q.astype(BF16)
    t["rT1k"] = T1k.astype(BF16); t["rT2k"] = T2k.astype(BF16)

    tt = np.arange(T)
    starts = np.arange(C) * STRIDE
    cm = np.where(tt[:, None] >= (starts + KERNEL - 1)[None, :], 0.0, NEG)
    t["cmask"] = cm.reshape(QT, 128, C).astype(BF16)
    t["valid"] = (tt >= KERNEL - 1).astype(np.float32).reshape(QT, 128, 1)

    qb = tt // BLOCK
    b = np.arange(NB)
    causal_b = b[None, :] <= qb[:, None]
    forced = (b[None, :] < 1) | (((qb[:, None] - b[None, :]) < 2) & causal_b)
    fadd = np.where(forced, 1000.0 + (NB - b)[None, :],
                    np.where(causal_b, 0.0, NEG))
    finv = np.where(forced, 0.0, 1.0)
    t["fadd"] = fadd.reshape(QT, 128, NB).astype(np.float32)
    t["finv"] = finv.reshape(QT, 128, NB).astype(BF16)

    r = np.arange(128)
    t["causal"] = np.where(r[:, None] >= r[None, :], 0.0, NEG).astype(BF16)
    t["rev"] = np.where(r[:, None] <= r[None, :], 0.0, NEG).astype(BF16)

    s_ = np.arange(T)
    t["E"] = (s_[None, :] // BLOCK == b[:, None]).astype(BF16)
    ov = ((starts[:, None] < (b * BLOCK + BLOCK)[None, :]) &
          ((starts + KERNEL)[:, None] > (b * BLOCK)[None, :])).astype(BF16)
    t["ov"] = ov
    _CACHE["tables"] = t
    return t


def _build_program():
    if "nc" in _CACHE:
        return _CACHE["nc"]
    import concourse.bacc as bacc
    import concourse.tile as tile
    import concourse.mybir as mybir
    import concourse.bass as bass
    from concourse.masks import make_identity
    from contextlib import ExitStack

    fp32 = mybir.dt.float32
    bf16 = mybir.dt.bfloat16
    AF = mybir.ActivationFunctionType
    OP = mybir.AluOpType
    AX = mybir.AxisListType
    ts = bass.ts

    nc = bacc.Bacc("TRN2", target_bir_lowering=False, debug=False)

    xT_d   = nc.dram_tensor("xT", [HID, T], bf16, kind="ExternalInput")
    wq_d   = nc.dram_tensor("WqTg", [HID, 1024], bf16, kind="ExternalInput")
    wkv_d  = nc.dram_tensor("WkvT", [HID, 128], bf16, kind="ExternalInput")
    wg_d   = nc.dram_tensor("WgT", [HID, 3], bf16, kind="ExternalInput")
    ck_d   = nc.dram_tensor("CKg", [KERNEL * D, D], bf16, kind="ExternalInput")
    cv_d   = nc.dram_tensor("CVg", [KERNEL * D, D], bf16, kind="ExternalInput")
    wo_d   = nc.dram_tensor("WoTc", [256, HID], bf16, kind="ExternalInput")
    rT1q_d = nc.dram_tensor("rT1q", [128, T], bf16, kind="ExternalInput")
    rT2q_d = nc.dram_tensor("rT2q", [128, T], bf16, kind="ExternalInput")
    rT1k_d = nc.dram_tensor("rT1k", [128, T], bf16, kind="ExternalInput")
    rT2k_d = nc.dram_tensor("rT2k", [128, T], bf16, kind="ExternalInput")
    cmask_d = nc.dram_tensor("cmask", [QT, 128, C], bf16, kind="ExternalInput")
    valid_d = nc.dram_tensor("valid", [QT, 128, 1], fp32, kind="ExternalInput")
    fadd_d = nc.dram_tensor("fadd", [QT, 128, NB], fp32, kind="ExternalInput")
    finv_d = nc.dram_tensor("finv", [QT, 128, NB], bf16, kind="ExternalInput")
    causal_d = nc.dram_tensor("causal", [128, 128], bf16, kind="ExternalInput")
    rev_d  = nc.dram_tensor("rev", [128, 128], bf16, kind="ExternalInput")
    E_d    = nc.dram_tensor("E", [NB, T], bf16, kind="ExternalInput")
    ov_d   = nc.dram_tensor("ov", [C, NB], bf16, kind="ExternalInput")
    o_d    = nc.dram_tensor("o_part", [T, HID], bf16, kind="ExternalOutput")

    with tile.TileContext(nc) as tc, ExitStack() as ctx:
        const = ctx.enter_context(tc.tile_pool(name="const", bufs=1))
        persist = ctx.enter_context(tc.tile_pool(name="persist", bufs=1))

        ident = const.tile([128, 128], bf16)
        make_identity(nc, ident[:])

        E_sb = const.tile([NB, T], bf16)
        nc.sync.dma_start(E_sb[:], E_d.ap())
        ov_sb = const.tile([C, NB], bf16)
        nc.sync.dma_start(ov_sb[:], ov_d.ap())
        causal_sb = const.tile([128, 128], bf16)
        nc.sync.dma_start(causal_sb[:], causal_d.ap())
        rev_sb = const.tile([128, 128], bf16)
        nc.sync.dma_start(rev_sb[:], rev_d.ap())
        cmask_sb = const.tile([128, QT, C], bf16)
        nc.sync.dma_start(cmask_sb[:], cmask_d.ap().rearrange("q p c -> p q c"))
        valid_sb = const.tile([128, QT], fp32)
        nc.sync.dma_start(valid_sb[:], valid_d.ap().rearrange("q p c -> p (q c)"))
        fadd_sb = const.tile([128, QT, NB], fp32)
        nc.sync.dma_start(fadd_sb[:], fadd_d.ap().rearrange("q p c -> p q c"))
        finv_sb = const.tile([128, QT, NB], bf16)
        nc.sync.dma_start(finv_sb[:], finv_d.ap().rearrange("q p c -> p q c"))
        wo_sb = const.tile([128, 2, HID], bf16)
        nc.sync.dma_start(wo_sb[:], wo_d.ap().rearrange("(c p) h -> p c h", p=128))

        gate_sb = persist.tile([128, QT, 3], fp32)
        ktvt = persist.tile([128, T], bf16)      # v rows 0-63, roped k rows 64-127
        kT0 = persist.tile([64, T], bf16)        # roped k at base partition 0
        qT = [persist.tile([128, T], bf16, tag=f"qT{m}", name=f"qT{m}")
              for m in range(8)]
        vnat = persist.tile([128, QT, D], bf16)  # v natural [T, 64]
        ckT = persist.tile([64, C], bf16)
        ckT64 = persist.tile([128, C], bf16)
        cvn = persist.tile([C, D], bf16)
        outT_all = persist.tile([128, 2, QT, 128], bf16)

        # ---------------- Phase A1: projections + rope ----------------
        with ExitStack() as actx:
            apool = actx.enter_context(tc.tile_pool(name="apool", bufs=1))
            astream = actx.enter_context(tc.tile_pool(name="astream", bufs=2))
            wqs = actx.enter_context(tc.tile_pool(name="wqs", bufs=4))
            ps_a = actx.enter_context(tc.tile_pool(name="ps_a", bufs=2, space="PSUM"))

            wkv_sb = apool.tile([128, 16, 128], bf16)
            nc.sync.dma_start(wkv_sb[:], wkv_d.ap().rearrange("(k p) m -> p k m", p=128))
            wg_sb = apool.tile([128, 16, 3], bf16)
            nc.sync.dma_start(wg_sb[:], wg_d.ap().rearrange("(k p) m -> p k m", p=128))
            rtq1 = apool.tile([128, T], bf16)
            nc.sync.dma_start(rtq1[:], rT1q_d.ap())
            rtq2 = apool.tile([128, T], bf16)
            nc.sync.dma_start(rtq2[:], rT2q_d.ap())
            rtk1 = apool.tile([128, T], bf16)
            nc.sync.dma_start(rtk1[:], rT1k_d.ap())
            rtk2 = apool.tile([128, T], bf16)
            nc.sync.dma_start(rtk2[:], rT2k_d.ap())

            wq_r = wq_d.ap().rearrange("(k p) m -> p k m", p=128)
            xT_r = xT_d.ap().rearrange("(k p) t -> p k t", p=128)

            def rope_q(raw, out, rt1, rt2, lo, hi, nq):
                sw = astream.tile([128, 512], bf16, tag="ropesw")
                for base in range(lo, hi, 64):
                    nc.sync.dma_start(sw[base:base+32, :], raw[base+32:base+64, :])
                    nc.sync.dma_start(sw[base+32:base+64, :], raw[base:base+32, :])
                m1 = astream.tile([128, 512], bf16, tag="ropem1")
                tsl = ts(nq, 512)
                nc.vector.tensor_tensor(out=m1[lo:hi, :], in0=raw[lo:hi, :],
                                        in1=rt1[lo:hi, tsl], op=OP.mult)
                nc.vector.tensor_tensor(out=sw[lo:hi, :], in0=sw[lo:hi, :],
                                        in1=rt2[lo:hi, tsl], op=OP.mult)
                nc.vector.tensor_tensor(out=out[lo:hi, tsl], in0=m1[lo:hi, :],
                                        in1=sw[lo:hi, :], op=OP.add)

            for nq in range(4):
                xh = apool.tile([128, 16, 512], bf16, tag="xh")
                nc.sync.dma_start(xh[:], xT_r[:, :, ts(nq, 512)])
                psum = ps_a.tile([128, 512], fp32, tag="pa")
                for k in range(16):
                    nc.tensor.matmul(psum[:], wkv_sb[:, k, :], xh[:, k, :],
                                     start=(k == 0), stop=(k == 15))
                kv_raw = astream.tile([128, 512], bf16, tag="kvraw")
                nc.scalar.copy(kv_raw[:], psum[:])
                nc.vector.tensor_copy(out=ktvt[0:64, ts(nq, 512)],
                                      in_=kv_raw[0:64, :])
                rope_q(kv_raw, ktvt, rtk1, rtk2, 64, 128, nq)
                for m in range(8):
                    psum = ps_a.tile([128, 512], fp32, tag="pa")
                    for k in range(16):
                        wqc = wqs.tile([128, 128], bf16, tag="wqc")
                        nc.sync.dma_start(wqc[:], wq_r[:, k, ts(m, 128)])
                        nc.tensor.matmul(psum[:], wqc[:], xh[:, k, :],
                                         start=(k == 0), stop=(k == 15))
                    q_raw = astream.tile([128, 512], bf16, tag="qraw")
                    nc.scalar.copy(q_raw[:], psum[:])
                    rope_q(q_raw, qT[m], rtq1, rtq2, 0, 128, nq)
                for qq in range(4):
                    q = nq * 4 + qq
                    psg = ps_a.tile([128, 3], fp32, tag="pg")
                    for k in range(16):
                        nc.tensor.matmul(psg[:], xh[:, k, ts(qq, 128)],
                                         wg_sb[:, k, :],
                                         start=(k == 0), stop=(k == 15))
                    nc.scalar.activation(gate_sb[:, q, :], psg[:], AF.Sigmoid)

            nc.sync.dma_start(kT0[:], ktvt[64:128, :])

        # ---------------- Phase A2: v natural + compression ----------------
        with ExitStack() as actx:
            a2 = actx.enter_context(tc.tile_pool(name="a2", bufs=2))
            ps_b = actx.enter_context(tc.tile_pool(name="ps_b", bufs=2, space="PSUM"))
            ps_c = actx.enter_context(tc.tile_pool(name="ps_c", bufs=1, space="PSUM"))

            for q in range(QT):
                pt = ps_b.tile([128, 128], bf16, tag="vt")
                nc.tensor.transpose(pt[:, 0:64], ktvt[0:64, ts(q, 128)],
                                    ident[0:64, 0:64])
                nc.vector.tensor_copy(out=vnat[:, q, :], in_=pt[:, 0:64])

            ckw_sb = a2.tile([128, 16, D], bf16, tag="ckw")
            nc.sync.dma_start(ckw_sb[:], ck_d.ap().rearrange("(k p) m -> p k m", p=128))
            cvw_sb = a2.tile([128, 16, D], bf16, tag="cvw")
            nc.sync.dma_start(cvw_sb[:], cv_d.ap().rearrange("(k p) m -> p k m", p=128))

            kview = ktvt[64:128, :].rearrange("d (c j) -> d c j", j=16)
            vview = ktvt[0:64, :].rearrange("d (c j) -> d c j", j=16)
            ck_ps = ps_c.tile([64, C], fp32, tag="ckps")
            cv_ps = ps_c.tile([64, C], fp32, tag="cvps")
            for m in range(16):
                kw = a2.tile([128, C], bf16, tag="kw")
                vw = a2.tile([128, C], bf16, tag="vw")
                for half in range(2):
                    j = 2 * m + half
                    co, jj = j // 16, j % 16
                    nc.sync.dma_start(kw[64*half:64*half+64, :], kview[:, co:co+C, jj])
                    nc.sync.dma_start(vw[64*half:64*half+64, :], vview[:, co:co+C, jj])
                nc.tensor.matmul(ck_ps[:], ckw_sb[:, m, :], kw[:],
                                 start=(m == 0), stop=(m == 15))
                nc.tensor.matmul(cv_ps[:], cvw_sb[:, m, :], vw[:],
                                 start=(m == 0), stop=(m == 15))
            nc.vector.tensor_copy(out=ckT[:], in_=ck_ps[:])
            nc.sync.dma_start(ckT64[64:128, :], ckT[:])
            cvT = a2.tile([64, C], bf16, tag="cvT")
            nc.vector.tensor_copy(out=cvT[:], in_=cv_ps[:])
            ptc = ps_b.tile([128, 128], bf16, tag="vt")
            nc.tensor.transpose(ptc[0:C, 0:64], cvT[:, 0:C], ident[0:64, 0:64])
            nc.vector.tensor_copy(out=cvn[:], in_=ptc[0:C, 0:64])

        # ---------------- Phases B + C per qtile ----------------
        work = ctx.enter_context(tc.tile_pool(name="work", bufs=2))
        small = ctx.enter_context(tc.tile_pool(name="small", bufs=4))
        pcomp_pool = ctx.enter_context(tc.tile_pool(name="pcomp", bufs=20))
        selp = ctx.enter_context(tc.tile_pool(name="selp", bufs=2))
        ps_s = ctx.enter_context(tc.tile_pool(name="ps_s", bufs=1, space="PSUM"))
        ps_pt = ctx.enter_context(tc.tile_pool(name="ps_pt", bufs=2, space="PSUM"))
        ps_o = ctx.enter_context(tc.tile_pool(name="ps_o", bufs=1, space="PSUM"))
        ps_m = ctx.enter_context(tc.tile_pool(name="ps_m", bufs=1, space="PSUM"))

        def qhead(h):
            return qT[h // 2], 64 * (h % 2)

        for q in range(QT):
            score_ps = ps_m.tile([128, NB], fp32, tag="score")
            pcomp = {}
            for h in range(16):
                qtile, base = qhead(h)
                cs = ps_s.tile([128, 512], fp32, tag="sch0")
                nc.tensor.matmul(cs[:, 0:C],
                                 qtile[base:base+64, ts(q, 128)],
                                 ckT64[64:128, :] if base else ckT[:],
                                 tile_position=(base, 0))
                nc.vector.tensor_tensor(out=cs[:, 0:C], in0=cs[:, 0:C],
                                        in1=cmask_sb[:, q, :], op=OP.add)
                mx = small.tile([128, 1], fp32, tag="b1mx")
                nc.vector.tensor_reduce(mx[:], cs[:, 0:C], axis=AX.X, op=OP.max)
                nmx = small.tile([128, 1], fp32, tag="b1nmx")
                nc.vector.tensor_scalar(out=nmx[:], in0=mx[:], scalar1=-1.0,
                                        scalar2=None, op0=OP.mult)
                e1 = small.tile([128, C], bf16, tag="b1e")
                sm = small.tile([128, 1], fp32, tag="b1sm")
                nc.scalar.activation(e1[:], cs[:, 0:C], AF.Exp, bias=nmx[:],
                                     accum_out=sm[:])
                ri = small.tile([128, 1], fp32, tag="b1ri")
                nc.vector.reciprocal(ri[:], sm[:])
                nc.vector.tensor_tensor(out=ri[:], in0=ri[:],
                                        in1=valid_sb[:, q:q+1], op=OP.mult)
                nc.vector.tensor_tensor(out=ri[:], in0=ri[:],
                                        in1=gate_sb[:, q, 0:1], op=OP.mult)
                nc.vector.tensor_scalar(out=e1[:], in0=e1[:], scalar1=ri[:],
                                        scalar2=None, op0=OP.mult)
                ptp = ps_pt.tile([128, 128], bf16, tag="pt")
                nc.tensor.transpose(ptp[0:C, :], e1[:, 0:C], ident[:])
                pc = pcomp_pool.tile([C, 128], bf16)
                nc.vector.tensor_copy(out=pc[:], in_=ptp[0:C, :])
                if h < 4:
                    pcomp[h] = pc
                nc.tensor.matmul(score_ps[:], pc[:], ov_sb[0:C, :],
                                 start=(h == 0), stop=(h == 15))

            sc = small.tile([128, NB], fp32, tag="sc")
            nc.vector.tensor_tensor(out=sc[:], in0=score_ps[:],
                                    in1=finv_sb[:, q, :], op=OP.mult)
            nc.vector.tensor_tensor(out=sc[:], in0=sc[:],
                                    in1=fadd_sb[:, q, :], op=OP.add)
            sce = small.tile([128, NB], fp32, tag="sce")
            nc.vector.tensor_copy(out=sce[:], in_=sc[:])
            for it in range(TOPK - 1):
                mx = small.tile([128, 1], fp32, tag="tkmx")
                nc.vector.tensor_reduce(mx[:], sce[:], axis=AX.X, op=OP.max)
                ge = small.tile([128, NB], fp32, tag="tkge")
                nc.vector.tensor_scalar(out=ge[:], in0=sce[:], scalar1=mx[:],
                                        scalar2=None, op0=OP.is_ge)
                nc.vector.scalar_tensor_tensor(out=sce[:], in0=ge[:],
                                               scalar=-2e30, in1=sce[:],
                                               op0=OP.mult, op1=OP.add)
            thr = small.tile([128, 1], fp32, tag="thr")
            nc.vector.tensor_reduce(thr[:], sce[:], axis=AX.X, op=OP.max)
            sel = small.tile([128, NB], fp32, tag="sel")
            nc.vector.tensor_scalar(out=sel[:], in0=sc[:], scalar1=thr[:],
                                    scalar2=None, op0=OP.is_ge)
            gt = small.tile([128, NB], fp32, tag="tkgt")
            nc.vector.tensor_scalar(out=gt[:], in0=sc[:], scalar1=-1e29,
                                    scalar2=None, op0=OP.is_gt)
            nc.vector.tensor_tensor(out=sel[:], in0=sel[:], in1=gt[:], op=OP.mult)
            seln = small.tile([128, NB], bf16, tag="seln")
            nc.vector.tensor_scalar(out=seln[:], in0=sel[:], scalar1=1.0,
                                    scalar2=1e30, op0=OP.subtract, op1=OP.mult)
            selt_ps = ps_pt.tile([128, 128], bf16, tag="pt")
            nc.tensor.transpose(selt_ps[0:NB, :], seln[:, 0:NB], ident[:])
            selT = selp.tile([NB, 128], bf16)
            nc.vector.tensor_copy(out=selT[:], in_=selt_ps[0:NB, :])

            kend = 128 * (q + 1)
            nch = (kend + 511) // 512
            w0t = max(0, q - 4)
            wspan = kend - 128 * w0t
            for h in range(4):
                qtile, base = qhead(h)
                lhs_q = qtile[base:base+64, ts(q, 128)]
                k_rhs = ktvt[64:128, :] if base else kT0[:]
                schs = []
                for ci in range(nch):
                    cw = min(512, kend - 512 * ci)
                    sch = ps_s.tile([128, 512], fp32, tag=f"sch{ci}")
                    nc.tensor.matmul(sch[:, 0:cw], lhs_q,
                                     k_rhs[:, 512*ci:512*ci+cw],
                                     tile_position=(base, 0))
                    schs.append(sch)
                dci, dof = (kend - 128) // 512, (kend - 128) % 512
                dsl = schs[dci][:, dof:dof+128]
                nc.vector.tensor_tensor(out=dsl, in0=dsl, in1=causal_sb[:],
                                        op=OP.add)

                def piece(kt):
                    ci, st = (128 * kt) // 512, (128 * kt) % 512
                    return schs[ci][:, st:st+128]

                e3 = work.tile([128, 640], bf16, tag="e3")
                revd = None
                if q >= 4:
                    revd = work.tile([128, 128], fp32, tag="revd")
                    nc.vector.tensor_tensor(out=revd[:], in0=piece(w0t),
                                            in1=rev_sb[:], op=OP.add)
                pieces = []
                for kt in range(w0t, q + 1):
                    ap = revd[:] if (q >= 4 and kt == w0t) else piece(kt)
                    pieces.append((ap, 128 * (kt - w0t)))
                m3 = small.tile([128, 1], fp32, tag="m3")
                for i, (ap, _) in enumerate(pieces):
                    pm = small.tile([128, 1], fp32, tag="m3p")
                    nc.vector.tensor_reduce(pm[:], ap, axis=AX.X, op=OP.max)
                    if i == 0:
                        nc.vector.tensor_copy(out=m3[:], in_=pm[:])
                    else:
                        nc.vector.tensor_tensor(out=m3[:], in0=m3[:], in1=pm[:],
                                                op=OP.max)
                nm3 = small.tile([128, 1], fp32, tag="nm3")
                nc.vector.tensor_scalar(out=nm3[:], in0=m3[:], scalar1=-1.0,
                                        scalar2=None, op0=OP.mult)
                s3 = small.tile([128, 1], fp32, tag="s3")
                for i, (ap, off) in enumerate(pieces):
                    psm = small.tile([128, 1], fp32, tag="psm3")
                    nc.scalar.activation(e3[:, off:off+128], ap, AF.Exp,
                                         bias=nm3[:], accum_out=psm[:])
                    if i == 0:
                        nc.vector.tensor_copy(out=s3[:], in_=psm[:])
                    else:
                        nc.vector.tensor_tensor(out=s3[:], in0=s3[:], in1=psm[:],
                                                op=OP.add)
                ri3 = small.tile([128, 1], fp32, tag="ri3")
                nc.vector.reciprocal(ri3[:], s3[:])
                nc.vector.tensor_tensor(out=ri3[:], in0=ri3[:],
                                        in1=gate_sb[:, q, 2:3], op=OP.mult)
                nc.vector.tensor_scalar(out=e3[:, 0:wspan], in0=e3[:, 0:wspan],
                                        scalar1=ri3[:], scalar2=None,
                                        op0=OP.mult)

                for ci in range(nch):
                    cw = min(512, kend - 512 * ci)
                    nc.tensor.matmul(schs[ci][:, 0:cw], selT[:],
                                     E_sb[:, 512*ci:512*ci+cw],
                                     start=False, stop=True,
                                     skip_group_check=True)
                m2 = small.tile([128, 1], fp32, tag="m2")
                for ci in range(nch):
                    cw = min(512, kend - 512 * ci)
                    pm = small.tile([128, 1], fp32, tag="m2p")
                    nc.vector.tensor_reduce(pm[:], schs[ci][:, 0:cw], axis=AX.X,
                                            op=OP.max)
                    if ci == 0:
                        nc.vector.tensor_copy(out=m2[:], in_=pm[:])
                    else:
                        nc.vector.tensor_tensor(out=m2[:], in0=m2[:], in1=pm[:],
                                                op=OP.max)
                nm2 = small.tile([128, 1], fp32, tag="nm2")
                nc.vector.tensor_scalar(out=nm2[:], in0=m2[:], scalar1=-1.0,
                                        scalar2=None, op0=OP.mult)
                e2 = work.tile([128, T], bf16, tag="e2")
                s2 = small.tile([128, 1], fp32, tag="s2")
                for ci in range(nch):
                    cw = min(512, kend - 512 * ci)
                    psm = small.tile([128, 1], fp32, tag="psm2")
                    nc.scalar.activation(e2[:, 512*ci:512*ci+cw],
                                         schs[ci][:, 0:cw], AF.Exp,
                                         bias=nm2[:], accum_out=psm[:])
                    if ci == 0:
                        nc.vector.tensor_copy(out=s2[:], in_=psm[:])
                    else:
                        nc.vector.tensor_tensor(out=s2[:], in0=s2[:], in1=psm[:],
                                                op=OP.add)
                ri2 = small.tile([128, 1], fp32, tag="ri2")
                nc.vector.reciprocal(ri2[:], s2[:])
                nc.vector.tensor_tensor(out=ri2[:], in0=ri2[:],
                                        in1=gate_sb[:, q, 1:2], op=OP.mult)
                if w0t > 0:
                    nc.vector.tensor_scalar(out=e2[:, 0:128*w0t],
                                            in0=e2[:, 0:128*w0t],
                                            scalar1=ri2[:], scalar2=None,
                                            op0=OP.mult)
                nc.vector.scalar_tensor_tensor(
                    out=e2[:, 128*w0t:kend], in0=e2[:, 128*w0t:kend],
                    scalar=ri2[:], in1=e3[:, 0:wspan], op0=OP.mult, op1=OP.add)

                acc = ps_o.tile([64, 128], fp32, tag="acc")
                nc.tensor.matmul(acc[:], cvn[:], pcomp[h][:],
                                 start=True, stop=False)
                for kt in range(q + 1):
                    ptp = ps_pt.tile([128, 128], bf16, tag="pt")
                    nc.tensor.transpose(ptp[:], e2[:, ts(kt, 128)], ident[:])
                    pts = work.tile([128, 128], bf16, tag="pts")
                    nc.vector.tensor_copy(out=pts[:], in_=ptp[:])
                    nc.tensor.matmul(acc[:], vnat[:, kt, :], pts[:],
                                     start=False, stop=(kt == q))
                otmp = work.tile([64, 128], bf16, tag="otmp")
                nc.vector.tensor_copy(out=otmp[:], in_=acc[:])
                nc.sync.dma_start(outT_all[base:base+64, h//2, q, :], otmp[:])

            osb = work.tile([128, HID], bf16, tag="osb")
            for od in range(4):
                ops_ = ps_s.tile([128, 512], fp32, tag="sch0")
                for hc in range(2):
                    nc.tensor.matmul(ops_[:], outT_all[:, hc, q, :],
                                     wo_sb[:, hc, ts(od, 512)],
                                     start=(hc == 0), stop=(hc == 1))
                nc.scalar.copy(osb[:, ts(od, 512)], ops_[:])
            nc.sync.dma_start(o_d.ap()[ts(q, 128), :], osb[:])

    nc.compile()
    _CACHE["nc"] = nc
    return nc


def _host_inputs(hidden_states, Wq, Wk, Wv, Wo, Wgate, compress_key, compress_value):
    t = _host_tables()
    x = np.asarray(hidden_states, np.float32)[0]
    xT = np.ascontiguousarray(x.T).astype(BF16)
    WqT = np.ascontiguousarray(np.asarray(Wq, np.float32).T)
    WkT = np.ascontiguousarray(np.asarray(Wk, np.float32).T)
    WvT = np.ascontiguousarray(np.asarray(Wv, np.float32).T)
    WgT = np.ascontiguousarray(np.asarray(Wgate, np.float32).T).astype(BF16)
    WoT = np.ascontiguousarray(np.asarray(Wo, np.float32).T)
    ckw = np.asarray(compress_key, np.float32)
    cvw = np.asarray(compress_value, np.float32)

    in_maps = []
    for c in range(8):
        g = c // 4
        own = [16 * g + 4 * (c % 4) + i for i in range(4)]
        rest = [16 * g + i for i in range(16) if 16 * g + i not in own]
        order = own + rest
        cols = np.concatenate([np.arange(h * D, (h + 1) * D) for h in order])
        wq = np.ascontiguousarray(WqT[:, cols]).astype(BF16)
        wkv = np.ascontiguousarray(
            np.concatenate([WvT[:, g*D:(g+1)*D], WkT[:, g*D:(g+1)*D]], axis=1)
        ).astype(BF16)
        wo_rows = np.concatenate([np.arange(h * D, (h + 1) * D) for h in own])
        in_maps.append({
            "xT": xT, "WqTg": wq, "WkvT": wkv, "WgT": WgT,
            "CKg": ckw[g].astype(BF16), "CVg": cvw[g].astype(BF16),
            "WoTc": np.ascontiguousarray(WoT[wo_rows]).astype(BF16),
            "rT1q": t["rT1q"], "rT2q": t["rT2q"],
            "rT1k": t["rT1k"], "rT2k": t["rT2k"],
            "cmask": t["cmask"], "valid": t["valid"],
            "fadd": t["fadd"], "finv": t["finv"],
            "causal": t["causal"], "rev": t["rev"],
            "E": t["E"], "ov": t["ov"],
        })
    return in_maps


def run(trace=False, **inputs):
    _install_ntff_hook()
    import concourse.bass_utils as bass_utils
    bass_utils.upload_artifacts = lambda d: d
    nc = _build_program()
    in_maps = _host_inputs(**inputs)
    res = bass_utils.run_bass_kernel_spmd(nc, in_maps, list(range(8)), trace=trace)
    o = np.zeros((T, HID), np.float64)
    for r in res.results:
        o += r["o_part"].astype(np.float64)
    return o.astype(np.float32)[None], res.exec_time_ns


def kernel(**inputs):
    out, _ = run(trace=False, **inputs)
    return out
